# revision 1
# baseline (speedup 1.0000x reference)
"""Bass kernel for nn_Attention_80393197847209 on trn2.

Strategy: batch-parallel over the 8 NeuronCores (B=8, one batch element per
core). All matmuls run as float32r (full f32 storage, reduced-precision
full-speed PE path). Big wq2/wk2 projections stream from HBM.
"""
import math
from contextlib import ExitStack

import numpy as np

import concourse.bacc as bacc
import concourse.mybir as mybir
import concourse.tile as tile
from concourse.masks import make_identity

P = 128
CL, QL, H, E2 = 512, 64, 768, 4608
CT_N = CL // P   # 4 c tiles
HT = H // P      # 6 h tiles
ET = E2 // P     # 36 e tiles
HD = 192         # head dim for both mha blocks
NHEAD1, NHEAD2 = 4, 24
NPAIR = NHEAD2 // 2  # head pairs in stage 2
ISQ = 1.0 / math.sqrt(HD)
NEG = -1e30
EPS = 1e-5

f32 = mybir.dt.float32
f32r = mybir.dt.float32r
EXP = mybir.ActivationFunctionType.Exp
SQRT = mybir.ActivationFunctionType.Sqrt
AX = mybir.AxisListType.X
MAX = mybir.AluOpType.max
MULT = mybir.AluOpType.mult
ADD = mybir.AluOpType.add
SUB = mybir.AluOpType.subtract

# x slice offsets: [c | a | c*a | c*b | scoat3 | acoat]
XO_C, XO_A, XO_CA, XO_CB, XO_S3, XO_AC = (i * H for i in range(6))


def _masked_softmax(nc, pool, src, out, m_b, nm_b, p, f, tag):
    """out = softmax over free dim of (src*m + nm), max-subtracted."""
    l = pool.tile([p, f], f32, tag=f"l_{tag}", name=f"l_{tag}")
    nc.vector.tensor_mul(l, src, m_b[0:p, 0:f])
    nc.vector.tensor_add(l, l, nm_b[0:p, 0:f])
    mx = pool.tile([p, 1], f32, tag=f"mx_{tag}", name=f"mx_{tag}")
    nc.vector.tensor_reduce(mx, l, axis=AX, op=MAX, negate=True)
    e = pool.tile([p, f], f32, tag=f"e_{tag}", name=f"e_{tag}")
    sm = pool.tile([p, 1], f32, tag=f"sm_{tag}", name=f"sm_{tag}")
    nc.scalar.activation(e, l, EXP, bias=mx, scale=1.0, accum_out=sm)
    r = pool.tile([p, 1], f32, tag=f"r_{tag}", name=f"r_{tag}")
    nc.vector.reciprocal(r, sm)
    nc.vector.tensor_scalar_mul(out, e, r)


def build(num_devices=8, debug=False):
    nc = bacc.Bacc("TRN2", target_bir_lowering=False, debug=False,
                   num_devices=num_devices)

    # ---- DRAM I/O ----
    d_c = nc.dram_tensor("c", (CL, H), f32r, kind="ExternalInput")
    d_q = nc.dram_tensor("q", (QL, H), f32r, kind="ExternalInput")
    d_cw = nc.dram_tensor("cw2", (H, 2), f32r, kind="ExternalInput")
    d_qw = nc.dram_tensor("qw2", (H, 2), f32r, kind="ExternalInput")
    d_cqw = nc.dram_tensor("cq_weight", (H,), f32, kind="ExternalInput")
    d_bias = nc.dram_tensor("bias", (1, 1), f32, kind="ExternalInput")
    d_wq1t = nc.dram_tensor("wq1t", (H, H), f32r, kind="ExternalInput")
    d_wk1t = nc.dram_tensor("wk1t", (H, H), f32r, kind="ExternalInput")
    d_bq1 = nc.dram_tensor("bq1", (H,), f32, kind="ExternalInput")
    d_bk1 = nc.dram_tensor("bk1", (H,), f32, kind="ExternalInput")
    d_gamma = nc.dram_tensor("gamma", (E2,), f32, kind="ExternalInput")
    d_beta = nc.dram_tensor("beta", (E2,), f32, kind="ExternalInput")
    d_wq2t = nc.dram_tensor("wq2t", (E2, E2), f32r, kind="ExternalInput")
    d_wk2t = nc.dram_tensor("wk2t", (E2, E2), f32r, kind="ExternalInput")
    d_bq2 = nc.dram_tensor("bq2", (E2,), f32, kind="ExternalInput")
    d_bk2 = nc.dram_tensor("bk2", (E2,), f32, kind="ExternalInput")
    d_qm = nc.dram_tensor("qm", (QL,), f32, kind="ExternalInput")
    d_nqm = nc.dram_tensor("nqm", (QL,), f32, kind="ExternalInput")
    d_cm = nc.dram_tensor("cm", (CL,), f32, kind="ExternalInput")
    d_ncm = nc.dram_tensor("ncm", (CL,), f32, kind="ExternalInput")
    d_out = nc.dram_tensor("out", (CL, E2), f32, kind="ExternalOutput")

    dbg = {}
    if debug:
        for name, shape in [("dbg_s", (QL, CL)), ("dbg_s2m", (QL, CL)),
                            ("dbg_scoat", (CL, QL)), ("dbg_x", (CL, E2)),
                            ("dbg_y", (CL, E2)), ("dbg_ss", (CL, CL)),
                            ("dbg_qh2t", (E2, CL))]:
            dbg[name] = nc.dram_tensor(name, shape, f32, kind="ExternalOutput")

    with tile.TileContext(nc) as tc, ExitStack() as es:
        const = es.enter_context(tc.tile_pool(name="const", bufs=1))
        dram = es.enter_context(tc.tile_pool(name="dram", bufs=1,
                                             space="DRAM"))
        trp = es.enter_context(tc.tile_pool(name="trp", bufs=2, space="PSUM"))

        # ---- constants / masks ----
        ident = const.tile([P, P], f32, tag="ident", name="ident")
        make_identity(nc, ident)
        cwT = const.tile([P, HT, 2], f32r, tag="cwT", name="cwT")
        nc.sync.dma_start(out=cwT,
                          in_=d_cw.ap().rearrange("(t p) k -> p t k", p=P))
        qwT = const.tile([P, HT, 2], f32r, tag="qwT", name="qwT")
        nc.sync.dma_start(out=qwT,
                          in_=d_qw.ap().rearrange("(t p) k -> p t k", p=P))
        cqwT = const.tile([P, HT], f32, tag="cqwT", name="cqwT")
        nc.sync.dma_start(out=cqwT,
                          in_=d_cqw.ap().rearrange("(t p) -> p t", p=P))
        bq1T = const.tile([P, HT], f32, tag="bq1T", name="bq1T")
        nc.sync.dma_start(out=bq1T,
                          in_=d_bq1.ap().rearrange("(t p) -> p t", p=P))
        bk1T = const.tile([P, HT], f32, tag="bk1T", name="bk1T")
        nc.sync.dma_start(out=bk1T,
                          in_=d_bk1.ap().rearrange("(t p) -> p t", p=P))
        bias_sb = const.tile([1, 1], f32, tag="bias", name="bias")
        nc.sync.dma_start(out=bias_sb, in_=d_bias[:, :])
        eps_sb = const.tile([P, 1], f32, tag="eps", name="eps")
        nc.vector.memset(eps_sb, EPS)

        xpark = dram.tile([CL, E2], f32)
        ypark = dram.tile([CL, E2], f32r)


        def pe_T(in_ap, pool=None):
            """PE transpose: returns PSUM AP [f, p] = in_ap.T (f32)."""
            p = in_ap.partition_size()
            f = in_ap.free_size()
            pst = (pool or trp).tile([P, P], f32, tag="tr", name="tr")
            out = pst[0:f, 0:p]
            nc.tensor.transpose(out, in_ap, ident[0:p, 0:p])
            return out

        # ================= stage 1 =================
        s1es = ExitStack()
        s1bes = ExitStack()
        with s1bes, s1es:
            s1b = s1bes.enter_context(tc.tile_pool(name="s1b", bufs=1))
            bigp = s1bes.enter_context(
                tc.tile_pool(name="bigp", bufs=1, space="PSUM"))
            s1a = s1es.enter_context(
                tc.tile_pool(name="s1a", bufs=1, side="right"))
            smallp = s1es.enter_context(
                tc.tile_pool(name="smallp", bufs=2, space="PSUM"))
            w1es = ExitStack()
            w1p = w1es.enter_context(
                tc.tile_pool(name="w1p", bufs=1, side="right"))

            crows = []
            for i in range(CT_N):
                t = s1b.tile([P, H], f32r, tag=f"crows{i}", name=f"crows{i}")
                nc.sync.dma_start(out=t, in_=d_c[i * P:(i + 1) * P, :])
                crows.append(t)
            qrows = s1b.tile([QL, H], f32r, tag="qrows", name="qrows")
            nc.sync.dma_start(out=qrows, in_=d_q[:, :])

            wq1t_sb, wk1t_sb = [], []
            for j in range(HT):
                t = w1p.tile([P, H], f32r, tag=f"wq1t{j}", name=f"wq1t{j}")
                nc.sync.dma_start(out=t, in_=d_wq1t[j * P:(j + 1) * P, :])
                wq1t_sb.append(t)
                t = w1p.tile([P, H], f32r, tag=f"wk1t{j}", name=f"wk1t{j}")
                nc.sync.dma_start(out=t, in_=d_wk1t[j * P:(j + 1) * P, :])
                wk1t_sb.append(t)

            qm_b = const.tile([P, QL], f32, tag="qm_b", name="qm_b")
            nc.sync.dma_start(out=qm_b, in_=d_qm.ap().partition_broadcast(P))
            nqm_b = const.tile([P, QL], f32, tag="nqm_b", name="nqm_b")
            nc.sync.dma_start(out=nqm_b, in_=d_nqm.ap().partition_broadcast(P))
            cm_b64 = const.tile([QL, CL], f32, tag="cm_b64", name="cm_b64")
            nc.sync.dma_start(out=cm_b64, in_=d_cm.ap().partition_broadcast(QL))
            ncm_b64 = const.tile([QL, CL], f32, tag="ncm_b64", name="ncm_b64")
            nc.sync.dma_start(out=ncm_b64, in_=d_ncm.ap().partition_broadcast(QL))
            # CT[j]: [128h, 512c], QT[j]: [128h, 64q]
            ct, qt = [], []
            for j in range(HT):
                tj = s1a.tile([P, CL], f32r, tag=f"ct{j}", name=f"ct{j}")
                for i in range(CT_N):
                    nc.vector.tensor_copy(
                        tj[:, i * P:(i + 1) * P],
                        pe_T(crows[i][:, j * P:(j + 1) * P].bitcast(f32)))
                ct.append(tj)
                qj = s1a.tile([P, QL], f32r, tag=f"qt{j}", name=f"qt{j}")
                nc.vector.tensor_copy(
                    qj, pe_T(qrows[:, j * P:(j + 1) * P].bitcast(f32)))
                qt.append(qj)

            # mha1 projections early (frees wq1t/wk1t)
            qh1T, kh1T = [], []
            for e in range(HT):
                ps = smallp.tile([P, CL], f32, tag="smA", name="qh1")
                for j in range(HT):
                    nc.tensor.matmul(ps, wq1t_sb[j][:, e * P:(e + 1) * P],
                                     ct[j], start=(j == 0),
                                     stop=(j == HT - 1))
                t = s1a.tile([P, CL], f32r, tag=f"qh1T{e}", name=f"qh1T{e}")
                nc.vector.tensor_scalar_add(t, ps, bq1T[:, e:e + 1])
                qh1T.append(t)
                ps = smallp.tile([P, QL], f32, tag="smB", name="kh1")
                for j in range(HT):
                    nc.tensor.matmul(ps, wk1t_sb[j][:, e * P:(e + 1) * P],
                                     qt[j], start=(j == 0),
                                     stop=(j == HT - 1))
                t = s1a.tile([P, QL], f32r, tag=f"kh1T{e}", name=f"kh1T{e}")
                nc.vector.tensor_scalar_add(t, ps, bk1T[:, e:e + 1])
                kh1T.append(t)
            w1es.close()

            # CWT[j] = CT[j] * cqw[j]
            cwt = []
            for j in range(HT):
                tj = s1a.tile([P, CL], f32r, tag=f"cwt{j}", name=f"cwt{j}")
                nc.vector.tensor_scalar_mul(tj, ct[j].bitcast(f32),
                                            cqwT[:, j:j + 1])
                cwt.append(tj)

            # ---- s matrices ----
            s0_ps = smallp.tile([2, CL], f32, tag="smA", name="s0")
            for j in range(HT):
                nc.tensor.matmul(s0_ps, cwT[:, j, :], ct[j],
                                 start=(j == 0), stop=(j == HT - 1))
            s1_ps = smallp.tile([2, QL], f32, tag="smB", name="s1c")
            for j in range(HT):
                nc.tensor.matmul(s1_ps, qwT[:, j, :], qt[j],
                                 start=(j == 0), stop=(j == HT - 1))

            # augmented K=1 operands: sT += s1row x ones + ones x (s0+bias)
            s1row = s1a.tile([1, QL], f32r, tag="s1row", name="s1row")
            nc.vector.tensor_copy(s1row, s1_ps[0:1, :])
            ones64 = s1a.tile([1, QL], f32r, tag="ones64", name="ones64")
            nc.vector.memset(ones64.bitcast(f32), 1.0)
            s0brow = s1a.tile([1, CL], f32r, tag="s0brow", name="s0brow")
            nc.vector.tensor_scalar_add(s0brow, s0_ps[0:1, :],
                                        bias_sb[0:1, :])
            ones512 = s1a.tile([1, CL], f32r, tag="ones512", name="ones512")
            nc.vector.memset(ones512.bitcast(f32), 1.0)

            sT_ps = smallp.tile([QL, CL], f32, tag="smA", name="sT")
            for j in range(HT):
                nc.tensor.matmul(sT_ps, qt[j], cwt[j], start=(j == 0),
                                 stop=False)
            nc.tensor.matmul(sT_ps, s1row, ones512, start=False, stop=False)
            nc.tensor.matmul(sT_ps, ones64, s0brow, start=False, stop=True)
            s_qc = s1a.tile([QL, CL], f32, tag="s_qc", name="s_qc")
            nc.vector.tensor_copy(s_qc, sT_ps)
            if dbg:
                nc.sync.dma_start(out=dbg["dbg_s"][:, :], in_=s_qc)

            # s2m in [q, c]
            s2m_qc = s1a.tile([QL, CL], f32r, tag="s2m_qc", name="s2m_qc")
            _masked_softmax(nc, s1a, s_qc, s2m_qc, cm_b64, ncm_b64, QL, CL,
                            "s2m")
            if dbg:
                nc.sync.dma_start(out=dbg["dbg_s2m"][:, :],
                                  in_=s2m_qc.bitcast(f32))

            # s1m in [c, q]
            s1m_cq = []
            for i in range(CT_N):
                sc = s1a.tile([P, QL], f32, tag=f"s_cq{i}", name=f"s_cq{i}")
                nc.vector.tensor_copy(sc, pe_T(s_qc[:, i * P:(i + 1) * P]))
                sm = s1a.tile([P, QL], f32, tag=f"s1m_cq{i}", name=f"s1m_cq{i}")
                _masked_softmax(nc, s1a, sc, sm, qm_b, nqm_b, P, QL,
                                f"s1m{i}")
                s1m_cq.append(sm)
            s1mT = s1b.tile([QL, CL], f32r, tag="s1mT", name="s1mT")
            for i in range(CT_N):
                nc.vector.tensor_copy(s1mT[:, i * P:(i + 1) * P],
                                      pe_T(s1m_cq[i]))

            # tT[d] [128d, 512c]
            tT_sb = []
            for d in range(CT_N):
                ps = smallp.tile([P, CL], f32, tag="smA", name="tT")
                nc.tensor.matmul(ps, s2m_qc[:, d * P:(d + 1) * P], s1mT,
                                 start=True, stop=True)
                t = s1b.tile([P, CL], f32r, tag=f"tT{d}", name=f"tT{d}")
                nc.vector.tensor_copy(t, ps)
                tT_sb.append(t)

            # ---- mha1 scores + scoat ----
            def _sub(tiles, src_j, lo, width, tag):
                t = s1a.tile([64, width], f32r, tag=tag)
                nc.vector.tensor_copy(t,
                                      tiles[src_j][lo:lo + 64, :].bitcast(f32))
                return t

            q_sub = {0: _sub(qh1T, 1, 0, CL, "qs0"),
                     1: _sub(qh1T, 1, 64, CL, "qs1"),
                     2: _sub(qh1T, 4, 0, CL, "qs2"),
                     3: _sub(qh1T, 4, 64, CL, "qs3")}
            k_sub = {0: _sub(kh1T, 1, 0, QL, "ks0"),
                     1: _sub(kh1T, 1, 64, QL, "ks1"),
                     2: _sub(kh1T, 4, 0, QL, "ks2"),
                     3: _sub(kh1T, 4, 64, QL, "ks3")}
            head_ops = {
                0: [(qh1T[0], kh1T[0]), (q_sub[0], k_sub[0])],
                1: [(q_sub[1], k_sub[1]), (qh1T[2], kh1T[2])],
                2: [(qh1T[3], kh1T[3]), (q_sub[2], k_sub[2])],
                3: [(q_sub[3], k_sub[3]), (qh1T[5], kh1T[5])],
            }

            scoat_cq = [s1a.tile([P, QL], f32, tag=f"scoat{i}", name=f"scoat{i}")
                        for i in range(CT_N)]
            for h in range(NHEAD1):
                for i in range(CT_N):
                    ps = smallp.tile([P, QL], f32, tag="smB", name="sc1")
                    ops = head_ops[h]
                    for ki, (ql, kr) in enumerate(ops):
                        nc.tensor.matmul(ps, ql[:, i * P:(i + 1) * P], kr,
                                         start=(ki == 0),
                                         stop=(ki == len(ops) - 1))
                    u = f"{h}_{i}"
                    mx = s1a.tile([P, 1], f32, tag=f"mx1{u}", name=f"mx1{u}")
                    nc.vector.tensor_reduce(mx, ps, axis=AX, op=MAX,
                                            negate=True)
                    mxs = s1a.tile([P, 1], f32, tag=f"mxs1{u}", name=f"mxs1{u}")
                    nc.vector.tensor_scalar_mul(mxs, mx, ISQ)
                    e_sb = s1a.tile([P, QL], f32, tag=f"e1{u}", name=f"e1{u}")
                    ssum = s1a.tile([P, 1], f32, tag=f"ssum1{u}", name=f"ssum1{u}")
                    nc.scalar.activation(e_sb, ps, EXP, bias=mxs, scale=ISQ,
                                         accum_out=ssum)
                    r = s1a.tile([P, 1], f32, tag=f"r1{u}", name=f"r1{u}")
                    nc.vector.reciprocal(r, ssum)
                    r4 = s1a.tile([P, 1], f32, tag=f"r41{u}", name=f"r41{u}")
                    nc.vector.tensor_scalar_mul(r4, r, 1.0 / NHEAD1)
                    if h == 0:
                        nc.vector.tensor_scalar_mul(scoat_cq[i], e_sb, r4)
                    else:
                        nc.vector.scalar_tensor_tensor(
                            scoat_cq[i], in0=e_sb, scalar=r4,
                            in1=scoat_cq[i], op0=MULT, op1=ADD)
            if dbg:
                for i in range(CT_N):
                    nc.sync.dma_start(
                        out=dbg["dbg_scoat"][i * P:(i + 1) * P, :],
                        in_=scoat_cq[i])

            # scoat1 -> scoat1T (f32r)
            scoat1T = s1b.tile([QL, CL], f32r, tag="scoat1T", name="scoat1T")
            for i in range(CT_N):
                sm = s1a.tile([P, QL], f32, tag=f"scoat1_{i}", name=f"scoat1_{i}")
                _masked_softmax(nc, s1a, scoat_cq[i], sm, qm_b, nqm_b, P, QL,
                                f"sc1_{i}")
                nc.vector.tensor_copy(scoat1T[:, i * P:(i + 1) * P],
                                      pe_T(sm))

            # scoatT -> scoat2_qc -> scoat2_cq (f32r)
            scoatT = s1a.tile([QL, CL], f32, tag="scoatT", name="scoatT")
            for i in range(CT_N):
                nc.vector.tensor_copy(scoatT[:, i * P:(i + 1) * P],
                                      pe_T(scoat_cq[i]))
            scoat2_qc = s1a.tile([QL, CL], f32, tag="scoat2_qc", name="scoat2_qc")
            _masked_softmax(nc, s1a, scoatT, scoat2_qc, cm_b64, ncm_b64,
                            QL, CL, "sc2")
            scoat2_cq = []
            for i in range(CT_N):
                t = s1a.tile([P, QL], f32r, tag=f"scoat2_cq{i}", name=f"scoat2_cq{i}")
                nc.vector.tensor_copy(t,
                                      pe_T(scoat2_qc[:, i * P:(i + 1) * P]))
                scoat2_cq.append(t)

            # bcoat [64q, 768h]
            bc_ps = bigp.tile([QL, H], f32, tag="big768", name="big768")
            for i in range(CT_N):
                nc.tensor.matmul(bc_ps[:, 0:512], scoat2_cq[i],
                                 crows[i][:, 0:512],
                                 start=(i == 0), stop=(i == CT_N - 1))
            for i in range(CT_N):
                nc.tensor.matmul(bc_ps[:, 512:H], scoat2_cq[i],
                                 crows[i][:, 512:H],
                                 start=(i == 0), stop=(i == CT_N - 1))
            bcoat = s1b.tile([QL, H], f32r, tag="bcoat", name="bcoat")
            nc.vector.tensor_copy(bcoat, bc_ps)
            s1es.close()  # free s1a pool, smallp
            trp2 = s1bes.enter_context(
                tc.tile_pool(name="trp2", bufs=4, space="PSUM"))

            ytp_es = ExitStack()
            ytp = ytp_es.enter_context(
                tc.tile_pool(name="ytp", bufs=1, side="right"))
            yT = [ytp.tile([P, CL], f32r, tag=f"yT{j}", name=f"yT{j}")
                  for j in range(ET)]

            # ---- per-c-tile x assembly + LN + park ----
            xsb_pool = s1bes.enter_context(tc.tile_pool(name="xsb", bufs=1))
            ysb_pool = s1bes.enter_context(tc.tile_pool(name="ysb", bufs=2))
            gb_pool = s1bes.enter_context(tc.tile_pool(name="gb", bufs=1))
            scr_pool = s1bes.enter_context(tc.tile_pool(name="scr", bufs=1))
            gamma_b = gb_pool.tile([P, E2], f32, tag="gamma_b", name="gamma_b")
            nc.sync.dma_start(out=gamma_b,
                              in_=d_gamma.ap().partition_broadcast(P))
            beta_b = gb_pool.tile([P, E2], f32, tag="beta_b", name="beta_b")
            nc.sync.dma_start(out=beta_b,
                              in_=d_beta.ap().partition_broadcast(P))

            pending_y = []
            for i in range(CT_N):
                x_i = xsb_pool.tile([P, E2], f32, tag="x", name="x")
                nc.vector.tensor_copy(x_i[:, XO_C:XO_C + H],
                                      crows[i].bitcast(f32))
                a_ps = bigp.tile([P, H], f32, tag="big768", name="big768")
                nc.tensor.matmul(a_ps[:, 0:512], s1mT[:, i * P:(i + 1) * P],
                                 qrows[:, 0:512], start=True, stop=True)
                nc.tensor.matmul(a_ps[:, 512:H], s1mT[:, i * P:(i + 1) * P],
                                 qrows[:, 512:H], start=True, stop=True)
                nc.scalar.copy(x_i[:, XO_A:XO_A + H], a_ps)
                nc.vector.tensor_mul(x_i[:, XO_CA:XO_CA + H],
                                     crows[i].bitcast(f32),
                                     x_i[:, XO_A:XO_A + H])
                b_ps = bigp.tile([P, H], f32, tag="big768", name="big768")
                for d in range(CT_N):
                    nc.tensor.matmul(b_ps[:, 0:512],
                                     tT_sb[d][:, i * P:(i + 1) * P],
                                     crows[d][:, 0:512],
                                     start=(d == 0), stop=(d == CT_N - 1))
                for d in range(CT_N):
                    nc.tensor.matmul(b_ps[:, 512:H],
                                     tT_sb[d][:, i * P:(i + 1) * P],
                                     crows[d][:, 512:H],
                                     start=(d == 0), stop=(d == CT_N - 1))
                b_sb = scr_pool.tile([P, H], f32, tag="b_sb", name="b_sb")
                nc.scalar.copy(b_sb, b_ps)
                nc.vector.tensor_mul(x_i[:, XO_CB:XO_CB + H],
                                     crows[i].bitcast(f32), b_sb)
                s3_ps = bigp.tile([P, H], f32, tag="big768", name="big768")
                nc.tensor.matmul(s3_ps[:, 0:512],
                                 scoat1T[:, i * P:(i + 1) * P],
                                 bcoat[:, 0:512], start=True, stop=True)
                nc.tensor.matmul(s3_ps[:, 512:H],
                                 scoat1T[:, i * P:(i + 1) * P],
                                 bcoat[:, 512:H], start=True, stop=True)
                nc.scalar.copy(x_i[:, XO_S3:XO_S3 + H], s3_ps)
                ac_ps = bigp.tile([P, H], f32, tag="big768", name="big768")
                nc.tensor.matmul(ac_ps[:, 0:512],
                                 scoat1T[:, i * P:(i + 1) * P],
                                 qrows[:, 0:512], start=True, stop=True)
                nc.tensor.matmul(ac_ps[:, 512:H],
                                 scoat1T[:, i * P:(i + 1) * P],
                                 qrows[:, 512:H], start=True, stop=True)
                nc.scalar.copy(x_i[:, XO_AC:XO_AC + H], ac_ps)

                # layernorm
                stats = scr_pool.tile([P, 9, 6], f32, tag="stats", name="stats")
                xg = x_i.rearrange("p (g d) -> p g d", g=9)
                for g in range(9):
                    nc.vector.bn_stats(out=stats[:, g, :], in_=xg[:, g, :])
                mv = scr_pool.tile([P, 2], f32, tag="mv", name="mv")
                nc.vector.bn_aggr(out=mv, in_=stats)
                rsq = scr_pool.tile([P, 1], f32, tag="rsq", name="rsq")
                nc.scalar.activation(rsq, mv[:, 1:2], SQRT, bias=eps_sb,
                                     scale=1.0)
                rstd = scr_pool.tile([P, 1], f32, tag="rstd", name="rstd")
                nc.vector.reciprocal(rstd, rsq)
                negmr = scr_pool.tile([P, 1], f32, tag="negmr", name="negmr")
                nc.vector.tensor_scalar(negmr, mv[:, 0:1], rstd, -1.0,
                                        op0=MULT, op1=MULT)
                y_i = ysb_pool.tile([P, E2], f32r, tag="y", name="y")
                yv = y_i.bitcast(f32)
                nc.scalar.activation(yv, x_i,
                                     mybir.ActivationFunctionType.Identity,
                                     bias=negmr, scale=rstd)
                nc.vector.tensor_mul(yv, yv, gamma_b)
                nc.vector.tensor_add(y_i, yv, beta_b)
                pending_y.append((i, y_i))
                if i > 0:
                    pi, py = pending_y.pop(0)
                    for j in range(ET):
                        nc.vector.tensor_copy(
                            yT[j][:, pi * P:(pi + 1) * P],
                            pe_T(py[:, j * P:(j + 1) * P].bitcast(f32),
                                 trp2))
                if dbg:
                    nc.sync.dma_start(out=dbg["dbg_x"][i * P:(i + 1) * P, :],
                                      in_=x_i)
                    nc.sync.dma_start(out=dbg["dbg_y"][i * P:(i + 1) * P, :],
                                      in_=y_i.bitcast(f32))
                nc.sync.dma_start(out=xpark[i * P:(i + 1) * P, :], in_=x_i)
                nc.sync.dma_start(out=ypark[i * P:(i + 1) * P, :], in_=y_i)
            for pi, py in pending_y:
                for j in range(ET):
                    nc.vector.tensor_copy(
                        yT[j][:, pi * P:(pi + 1) * P],
                        pe_T(py[:, j * P:(j + 1) * P].bitcast(f32), trp2))
        # stage-1 pools all freed

        # ================= phase 6: projections + scores + ss ========
        p56 = ExitStack()
        ssp = es.enter_context(tc.tile_pool(name="ssp", bufs=1))
        ss = [ssp.tile([P, CL], f32, tag=f"ss{i}", name=f"ss{i}") for i in range(CT_N)]
        with p56:
            wst = p56.enter_context(tc.tile_pool(name="wst", bufs=7))
            prp = p56.enter_context(tc.tile_pool(name="prp", bufs=2))
            prps = p56.enter_context(
                tc.tile_pool(name="prps", bufs=3, space="PSUM"))
            scps = p56.enter_context(
                tc.tile_pool(name="scps", bufs=3, space="PSUM"))
            smp = p56.enter_context(tc.tile_pool(name="smp", bufs=4))

            bq2T = const.tile([P, ET], f32, tag="bq2T", name="bq2T")
            nc.sync.dma_start(out=bq2T,
                                      in_=d_bq2.ap().rearrange("(t p) -> p t", p=P))
            bk2T = const.tile([P, ET], f32, tag="bk2T", name="bk2T")
            nc.sync.dma_start(out=bk2T,
                                      in_=d_bk2.ap().rearrange("(t p) -> p t", p=P))
            CH = 6
            NCHUNK = ET // CH
            for pair in range(NPAIR):
                e0 = pair * 384
                projT = {}
                for side, dw, bT in (("q", d_wq2t, bq2T),
                                     ("k", d_wk2t, bk2T)):
                    chunks = []
                    for cki in range(NCHUNK):
                        wt = wst.tile([P, CH, 384], f32r, tag="wchunk", name="wchunk")
                        src = dw.ap()[cki * CH * P:(cki + 1) * CH * P,
                                      e0:e0 + 384]
                        nc.sync.dma_start(
                            out=wt, in_=src.rearrange("(t p) e -> p t e",
                                                      p=P))
                        chunks.append(wt)
                    pss = [prps.tile([P, CL], f32, tag=f"proj{e_}",
                                     name=f"proj{e_}", bufs=1)
                           for e_ in range(3)]
                    for j in range(ET):
                        wt = chunks[j // CH]
                        for esub in range(3):
                            nc.tensor.matmul(
                                pss[esub],
                                wt[:, j % CH, esub * P:(esub + 1) * P],
                                yT[j], start=(j == 0), stop=(j == ET - 1))
                    outs = []
                    for esub in range(3):
                        et_idx = (e0 // P) + esub
                        t = prp.tile([P, CL], f32r, tag=f"projT_{side}{esub}",
                                     name=f"projT_{side}{esub}", bufs=1)
                        nc.vector.tensor_scalar_add(
                            t, pss[esub], bT[:, et_idx:et_idx + 1])
                        outs.append(t)
                    lo = prp.tile([64, CL], f32r, tag=f"projlo{side}",
                                  name=f"projlo{side}", bufs=1)
                    nc.vector.tensor_copy(lo, outs[1][0:64, :].bitcast(f32))
                    hi = prp.tile([64, CL], f32r, tag=f"projhi{side}",
                                  name=f"projhi{side}", bufs=1)
                    nc.vector.tensor_copy(hi, outs[1][64:P, :].bitcast(f32))
                    projT[side] = (outs, lo, hi)
                    if dbg and side == "q":
                        for esub in range(3):
                            nc.sync.dma_start(
                                out=dbg["dbg_qh2t"][
                                    e0 + esub * P:e0 + (esub + 1) * P, :],
                                in_=outs[esub].bitcast(f32))

                qo, qlo, qhi = projT["q"]
                ko, klo, khi = projT["k"]
                for hh in range(2):
                    if hh == 0:
                        kops = [(qo[0], ko[0]), (qlo, klo)]
                    else:
                        kops = [(qhi, khi), (qo[2], ko[2])]
                    head_idx = pair * 2 + hh
                    for i in range(CT_N):
                        ps = scps.tile([P, CL], f32, tag="sc2", name="sc2")
                        for ki, (ql, kr) in enumerate(kops):
                            nc.tensor.matmul(ps, ql[:, i * P:(i + 1) * P],
                                             kr, start=(ki == 0),
                                             stop=(ki == 1))
                        mx = smp.tile([P, 1], f32, tag=f"mx2_{i}", name=f"mx2_{i}")
                        nc.vector.tensor_reduce(mx, ps, axis=AX, op=MAX,
                                                negate=True)
                        mxs = smp.tile([P, 1], f32, tag=f"mxs2_{i}", name=f"mxs2_{i}")
                        nc.vector.tensor_scalar_mul(mxs, mx, ISQ)
                        e_sb = smp.tile([P, CL], f32, tag=f"e2_{i}",
                                        name=f"e2_{i}", bufs=2)
                        ssum = smp.tile([P, 1], f32, tag=f"ssum2_{i}", name=f"ssum2_{i}")
                        nc.scalar.activation(e_sb, ps, EXP, bias=mxs,
                                             scale=ISQ, accum_out=ssum)
                        r = smp.tile([P, 1], f32, tag=f"r2_{i}", name=f"r2_{i}")
                        nc.vector.reciprocal(r, ssum)
                        r24 = smp.tile([P, 1], f32, tag=f"r242_{i}", name=f"r242_{i}")
                        nc.vector.tensor_scalar_mul(r24, r, 1.0 / NHEAD2)
                        if head_idx == 0:
                            nc.vector.tensor_scalar_mul(ss[i], e_sb, r24)
                        else:
                            nc.vector.scalar_tensor_tensor(
                                ss[i], in0=e_sb, scalar=r24,
                                in1=ss[i], op0=MULT, op1=ADD)

            if dbg:
                for i in range(CT_N):
                    nc.sync.dma_start(out=dbg["dbg_ss"][i * P:(i + 1) * P, :],
                                      in_=ss[i])
        # yT, weight stream pools freed

        ytp_es.close()

        # ================= phase 7: ss1 + patt =================
        with ExitStack() as f7:
            fin = f7.enter_context(tc.tile_pool(name="fin", bufs=1))
            xre = f7.enter_context(tc.tile_pool(name="xre", bufs=2))
            outp = f7.enter_context(tc.tile_pool(name="outp", bufs=3))
            pps = f7.enter_context(
                tc.tile_pool(name="pps", bufs=3, space="PSUM"))

            cm_b128 = const.tile([P, CL], f32, tag="cm_b128", name="cm_b128")
            nc.sync.dma_start(out=cm_b128, in_=d_cm.ap().partition_broadcast(P))
            ncm_b128 = const.tile([P, CL], f32, tag="ncm_b128", name="ncm_b128")
            nc.sync.dma_start(out=ncm_b128, in_=d_ncm.ap().partition_broadcast(P))
            y_sb = []
            for d in range(CT_N):
                t = fin.tile([P, E2], f32r, tag=f"yf{d}", name=f"yf{d}")
                nc.sync.dma_start(out=t, in_=ypark[d * P:(d + 1) * P, :])
                y_sb.append(t)

            ss1T = []
            for d in range(CT_N):
                sst = fin.tile([P, CL], f32, tag=f"ssT{d}", name=f"ssT{d}")
                for i in range(CT_N):
                    nc.vector.tensor_copy(sst[:, i * P:(i + 1) * P],
                                          pe_T(ss[i][:, d * P:(d + 1) * P]))
                t = fin.tile([P, CL], f32r, tag=f"ss1T{d}", name=f"ss1T{d}")
                _masked_softmax(nc, fin, sst, t, cm_b128, ncm_b128, P, CL,
                                f"ss1_{d}")
                ss1T.append(t)

            x_re = []
            for i in range(CT_N):
                t = xre.tile([P, E2], f32, tag=f"xf{i}", name=f"xf{i}",
                             bufs=1)
                nc.sync.dma_start(out=t, in_=xpark[i * P:(i + 1) * P, :])
                x_re.append(t)
            for i in range(CT_N):
                x_i = x_re[i]
                for hs in range(E2 // 512):
                    ps = pps.tile([P, 512], f32, tag="patt", name="patt")
                    for d in range(CT_N):
                        nc.tensor.matmul(
                            ps, ss1T[d][:, i * P:(i + 1) * P],
                            y_sb[d][:, hs * 512:(hs + 1) * 512],
                            start=(d == 0), stop=(d == CT_N - 1))
                    o = outp.tile([P, 512], f32, tag="out", name="out")
                    nc.vector.tensor_add(o, ps,
                                         x_i[:, hs * 512:(hs + 1) * 512])
                    nc.sync.dma_start(
                        out=d_out[i * P:(i + 1) * P,
                                  hs * 512:(hs + 1) * 512],
                        in_=o)

    nc.compile()
    return nc


# ================= host side =================

_CACHE = {}


def prep_shared(inputs):
    f = np.float32
    cw2 = np.zeros((768, 2), f)
    cw2[:, 0] = np.asarray(inputs["c_weight"], f).reshape(-1)
    qw2 = np.zeros((768, 2), f)
    qw2[:, 0] = np.asarray(inputs["q_weight"], f).reshape(-1)
    return {
        "cw2": cw2,
        "qw2": qw2,
        "cq_weight": np.ascontiguousarray(
            np.asarray(inputs["cq_weight"], f).reshape(-1)),
        "bias": np.ascontiguousarray(
            np.asarray(inputs["bias"], f).reshape(1, 1)),
        "wq1t": np.ascontiguousarray(np.asarray(inputs["wq1"], f).T),
        "wk1t": np.ascontiguousarray(np.asarray(inputs["wk1"], f).T),
        "bq1": np.ascontiguousarray(np.asarray(inputs["bq1"], f)),
        "bk1": np.ascontiguousarray(np.asarray(inputs["bk1"], f)),
        "gamma": np.ascontiguousarray(np.asarray(inputs["gamma"], f)),
        "beta": np.ascontiguousarray(np.asarray(inputs["beta"], f)),
        "wq2t": np.ascontiguousarray(np.asarray(inputs["wq2"], f).T),
        "wk2t": np.ascontiguousarray(np.asarray(inputs["wk2"], f).T),
        "bq2": np.ascontiguousarray(np.asarray(inputs["bq2"], f)),
        "bk2": np.ascontiguousarray(np.asarray(inputs["bk2"], f)),
    }


def make_in_maps(inputs, n_cores=8):
    f = np.float32
    shared = prep_shared(inputs)
    c = np.asarray(inputs["c"], f)
    q = np.asarray(inputs["q"], f)
    cm = np.asarray(inputs["c_mask"], f)
    qm = np.asarray(inputs["q_mask"], f)
    in_maps = []
    for b in range(n_cores):
        m = dict(shared)
        m["c"] = np.ascontiguousarray(c[b])
        m["q"] = np.ascontiguousarray(q[b])
        m["cm"] = np.ascontiguousarray(cm[b])
        m["ncm"] = np.ascontiguousarray((1.0 - cm[b]) * np.float32(NEG))
        m["qm"] = np.ascontiguousarray(qm[b])
        m["nqm"] = np.ascontiguousarray((1.0 - qm[b]) * np.float32(NEG))
        in_maps.append(m)
    return in_maps


def kernel(**inputs):
    from concourse.bass_utils import run_bass_kernel_spmd

    B = inputs["c"].shape[0]
    if "nc" not in _CACHE:
        _CACHE["nc"] = build(num_devices=B)
    nc = _CACHE["nc"]
    in_maps = make_in_maps(inputs, B)
    res = run_bass_kernel_spmd(nc, in_maps, core_ids=list(range(B)))
    out = np.stack([res.results[b]["out"] for b in range(B)])
    return out



# revision 5
# speedup vs baseline: 1.4739x; 1.4739x over previous
"""Bass kernel for nn_Attention_80393197847209 on trn2.

Strategy: batch-parallel over the 8 NeuronCores (B=8, one batch element per
core). Stage-1 matmuls run as float32r. The dominant stage-2 QK projections
(y @ wq2^T, y @ wk2^T with 4608x4608 weights) run as fp8 e4m3 DoubleRow
matmuls (2x PE rate, 4x less weight DMA); scores and the final attention
matmul run in bf16. x and y stay resident in SBUF as bf16 (no DRAM
round-trip). Softmaxes skip max-subtraction (logit ranges are safe in f32)
and fold the 1/nheads scaling into masks / the transpose copy.
"""
import math
from contextlib import ExitStack

import numpy as np

import concourse.bacc as bacc
import concourse.mybir as mybir
import concourse.tile as tile
from concourse.masks import make_identity

P = 128
CL, QL, H, E2 = 512, 64, 768, 4608
CT_N = CL // P   # 4 c tiles
HT = H // P      # 6 h tiles
ET = E2 // P     # 36 e tiles
HD = 192         # head dim for both mha blocks
NHEAD1, NHEAD2 = 4, 24
NPAIR = NHEAD2 // 2  # head pairs in stage 2
CH = 6               # k-tiles per weight chunk
NCHUNK = ET // CH    # 6 chunks per (pair, side)
ISQ = 1.0 / math.sqrt(HD)
NEG = -1e30
EPS = 1e-5

f32 = mybir.dt.float32
f32r = mybir.dt.float32r
bf16 = mybir.dt.bfloat16
f8 = mybir.dt.float8e4
DR = mybir.MatmulPerfMode.DoubleRow
EXP = mybir.ActivationFunctionType.Exp
SQRT = mybir.ActivationFunctionType.Sqrt
IDENT = mybir.ActivationFunctionType.Identity
AX = mybir.AxisListType.X
MAX = mybir.AluOpType.max
MULT = mybir.AluOpType.mult
ADD = mybir.AluOpType.add

# x slice offsets: [c | a | c*a | c*b | scoat3 | acoat]
XO_C, XO_A, XO_CA, XO_CB, XO_S3, XO_AC = (i * H for i in range(6))


def _msoftmax(nc, pool, src, out, m_b, nm_b, p, f, tag):
    """out = softmax over free dim of (src*m + nm); no max-subtraction."""
    l = pool.tile([p, f], f32, tag=f"l_{tag}", name=f"l_{tag}")
    nc.vector.tensor_mul(l, src, m_b[0:p, 0:f])
    nc.vector.tensor_add(l, l, nm_b[0:p, 0:f])
    e = pool.tile([p, f], f32, tag=f"e_{tag}", name=f"e_{tag}")
    sm = pool.tile([p, 1], f32, tag=f"sm_{tag}", name=f"sm_{tag}")
    nc.scalar.activation(e, l, EXP, accum_out=sm)
    r = pool.tile([p, 1], f32, tag=f"r_{tag}", name=f"r_{tag}")
    nc.vector.reciprocal(r, sm)
    nc.vector.tensor_scalar_mul(out, e, r)


def build(num_devices=8):
    nc = bacc.Bacc("TRN2", target_bir_lowering=False, debug=False,
                   num_devices=num_devices)

    # ---- DRAM I/O ----
    d_c = nc.dram_tensor("c", (CL, H), f32r, kind="ExternalInput")
    d_q = nc.dram_tensor("q", (QL, H), f32r, kind="ExternalInput")
    d_cw = nc.dram_tensor("cw2", (H, 2), f32r, kind="ExternalInput")
    d_qw = nc.dram_tensor("qw2", (H, 2), f32r, kind="ExternalInput")
    d_cqw = nc.dram_tensor("cq_weight", (H,), f32, kind="ExternalInput")
    d_bias = nc.dram_tensor("bias", (1, 1), f32, kind="ExternalInput")
    d_wq1t = nc.dram_tensor("wq1t", (H, H), f32r, kind="ExternalInput")
    d_wk1t = nc.dram_tensor("wk1t", (H, H), f32r, kind="ExternalInput")
    d_bq1 = nc.dram_tensor("bq1", (H,), f32, kind="ExternalInput")
    d_bk1 = nc.dram_tensor("bk1", (H,), f32, kind="ExternalInput")
    d_gamma = nc.dram_tensor("gammab", (E2,), bf16, kind="ExternalInput")
    d_beta = nc.dram_tensor("betab", (E2,), bf16, kind="ExternalInput")
    # fp8 stage-2 weights, tiled [pair, cki, p, t, e] with k=(cki*6+t)*128+p
    d_wq2t8 = nc.dram_tensor("wq2t8", (NPAIR * NCHUNK * P * CH, 384), f8,
                             kind="ExternalInput")
    d_wk2t8 = nc.dram_tensor("wk2t8", (NPAIR * NCHUNK * P * CH, 384), f8,
                             kind="ExternalInput")
    d_bq2 = nc.dram_tensor("bq2", (E2,), f32, kind="ExternalInput")
    d_bk2 = nc.dram_tensor("bk2", (E2,), f32, kind="ExternalInput")
    d_qm = nc.dram_tensor("qm", (QL,), f32, kind="ExternalInput")
    d_nqm = nc.dram_tensor("nqm", (QL,), f32, kind="ExternalInput")
    d_qm4 = nc.dram_tensor("qm4", (QL,), f32, kind="ExternalInput")
    d_cm = nc.dram_tensor("cm", (CL,), f32, kind="ExternalInput")
    d_ncm = nc.dram_tensor("ncm", (CL,), f32, kind="ExternalInput")
    d_cm4 = nc.dram_tensor("cm4", (CL,), f32, kind="ExternalInput")
    d_out = nc.dram_tensor("out", (CL, E2), f32, kind="ExternalOutput")

    with tile.TileContext(nc) as tc, ExitStack() as es:
        const = es.enter_context(tc.tile_pool(name="const", bufs=1))
        trp = es.enter_context(tc.tile_pool(name="trp", bufs=2, space="PSUM"))
        wst = es.enter_context(tc.tile_pool(name="wst", bufs=16))

        # ---- constants / masks ----
        ident = const.tile([P, P], f32, tag="ident", name="ident")
        make_identity(nc, ident)
        identb = const.tile([P, P], bf16, tag="identb", name="identb")
        nc.vector.tensor_copy(identb, ident)
        cwT = const.tile([P, HT, 2], f32r, tag="cwT", name="cwT")
        nc.sync.dma_start(out=cwT,
                          in_=d_cw.ap().rearrange("(t p) k -> p t k", p=P))
        qwT = const.tile([P, HT, 2], f32r, tag="qwT", name="qwT")
        nc.sync.dma_start(out=qwT,
                          in_=d_qw.ap().rearrange("(t p) k -> p t k", p=P))
        cqwT = const.tile([P, HT], f32, tag="cqwT", name="cqwT")
        nc.sync.dma_start(out=cqwT,
                          in_=d_cqw.ap().rearrange("(t p) -> p t", p=P))
        bq1T = const.tile([P, HT], f32, tag="bq1T", name="bq1T")
        nc.sync.dma_start(out=bq1T,
                          in_=d_bq1.ap().rearrange("(t p) -> p t", p=P))
        bk1T = const.tile([P, HT], f32, tag="bk1T", name="bk1T")
        nc.sync.dma_start(out=bk1T,
                          in_=d_bk1.ap().rearrange("(t p) -> p t", p=P))
        bias_sb = const.tile([1, 1], f32, tag="bias", name="bias")
        nc.sync.dma_start(out=bias_sb, in_=d_bias[:, :])
        eps_sb = const.tile([P, 1], f32, tag="eps", name="eps")
        nc.vector.memset(eps_sb, EPS)

        # stage-2 weight chunk prefetch machinery
        w_chunks = {}

        def load_pair_chunks(pair):
            if pair >= NPAIR or pair in w_chunks:
                return
            by_side = {}
            for side, dw in (("q", d_wq2t8), ("k", d_wk2t8)):
                chunks = []
                for cki in range(NCHUNK):
                    wt = wst.tile([P, CH, 384], f8, tag="wchunk",
                                  name="wchunk")
                    base = (pair * NCHUNK + cki) * P * CH
                    src = dw.ap()[base:base + P * CH, :]
                    nc.sync.dma_start(
                        out=wt, in_=src.rearrange("(p t) e -> p t e", p=P))
                    chunks.append(wt)
                by_side[side] = chunks
            w_chunks[pair] = by_side

        def pe_T(in_ap, pool=None):
            """PE transpose: returns PSUM AP [f, p] = in_ap.T (f32)."""
            p = in_ap.partition_size()
            f = in_ap.free_size()
            pst = (pool or trp).tile([P, P], f32, tag="tr", name="tr")
            out = pst[0:f, 0:p]
            nc.tensor.transpose(out, in_ap, ident[0:p, 0:p])
            return out

        def pe_Tb(in_ap, pool):
            """PE transpose of a bf16 tile -> PSUM bf16 [f, p]."""
            p = in_ap.partition_size()
            f = in_ap.free_size()
            pst = pool.tile([P, P], bf16, tag="trb", name="trb")
            out = pst[0:f, 0:p]
            nc.tensor.transpose(out, in_ap, identb[0:p, 0:p])
            return out

        # ================= stage 1 =================
        s1bes = ExitStack()
        s1es = ExitStack()
        with s1bes, s1es:
            s1b = s1bes.enter_context(tc.tile_pool(name="s1b", bufs=1))
            s1a = s1es.enter_context(
                tc.tile_pool(name="s1a", bufs=1, side="right"))
            smallp = s1es.enter_context(
                tc.tile_pool(name="smallp", bufs=2, space="PSUM"))
            w1es = ExitStack()
            w1p = w1es.enter_context(
                tc.tile_pool(name="w1p", bufs=1, side="right"))

            crows = []
            for i in range(CT_N):
                t = s1b.tile([P, H], f32r, tag=f"crows{i}", name=f"crows{i}")
                nc.sync.dma_start(out=t, in_=d_c[i * P:(i + 1) * P, :])
                crows.append(t)
            qrows = s1b.tile([QL, H], f32r, tag="qrows", name="qrows")
            nc.sync.dma_start(out=qrows, in_=d_q[:, :])

            wq1t_sb, wk1t_sb = [], []
            for j in range(HT):
                t = w1p.tile([P, H], f32r, tag=f"wq1t{j}", name=f"wq1t{j}")
                nc.sync.dma_start(out=t, in_=d_wq1t[j * P:(j + 1) * P, :])
                wq1t_sb.append(t)
                t = w1p.tile([P, H], f32r, tag=f"wk1t{j}", name=f"wk1t{j}")
                nc.sync.dma_start(out=t, in_=d_wk1t[j * P:(j + 1) * P, :])
                wk1t_sb.append(t)

            qm_b = const.tile([P, QL], f32, tag="qm_b", name="qm_b")
            nc.sync.dma_start(out=qm_b, in_=d_qm.ap().partition_broadcast(P))
            nqm_b = const.tile([P, QL], f32, tag="nqm_b", name="nqm_b")
            nc.sync.dma_start(out=nqm_b, in_=d_nqm.ap().partition_broadcast(P))
            qm_b4 = const.tile([P, QL], f32, tag="qm_b4", name="qm_b4")
            nc.sync.dma_start(out=qm_b4, in_=d_qm4.ap().partition_broadcast(P))
            cm_b64 = const.tile([QL, CL], f32, tag="cm_b64", name="cm_b64")
            nc.sync.dma_start(out=cm_b64, in_=d_cm.ap().partition_broadcast(QL))
            ncm_b64 = const.tile([QL, CL], f32, tag="ncm_b64", name="ncm_b64")
            nc.sync.dma_start(out=ncm_b64,
                              in_=d_ncm.ap().partition_broadcast(QL))
            cm_b64_4 = const.tile([QL, CL], f32, tag="cm_b64_4",
                                  name="cm_b64_4")
            nc.sync.dma_start(out=cm_b64_4,
                              in_=d_cm4.ap().partition_broadcast(QL))

            # prefetch stage-2 weights for the first two pairs during stage 1
            load_pair_chunks(0)
            load_pair_chunks(1)

            # CT[j]: [128h, 512c], QT[j]: [128h, 64q]
            ct, qt = [], []
            for j in range(HT):
                tj = s1a.tile([P, CL], f32r, tag=f"ct{j}", name=f"ct{j}")
                for i in range(CT_N):
                    nc.vector.tensor_copy(
                        tj[:, i * P:(i + 1) * P],
                        pe_T(crows[i][:, j * P:(j + 1) * P].bitcast(f32)))
                ct.append(tj)
                qj = s1a.tile([P, QL], f32r, tag=f"qt{j}", name=f"qt{j}")
                nc.vector.tensor_copy(
                    qj, pe_T(qrows[:, j * P:(j + 1) * P].bitcast(f32)))
                qt.append(qj)

            # mha1 projections early (frees wq1t/wk1t)
            qh1T, kh1T = [], []
            for e in range(HT):
                ps = smallp.tile([P, CL], f32, tag="smA", name="qh1")
                for j in range(HT):
                    nc.tensor.matmul(ps, wq1t_sb[j][:, e * P:(e + 1) * P],
                                     ct[j], start=(j == 0),
                                     stop=(j == HT - 1))
                t = s1a.tile([P, CL], f32r, tag=f"qh1T{e}", name=f"qh1T{e}")
                nc.vector.tensor_scalar_add(t, ps, bq1T[:, e:e + 1])
                qh1T.append(t)
                ps = smallp.tile([P, QL], f32, tag="smB", name="kh1")
                for j in range(HT):
                    nc.tensor.matmul(ps, wk1t_sb[j][:, e * P:(e + 1) * P],
                                     qt[j], start=(j == 0),
                                     stop=(j == HT - 1))
                t = s1a.tile([P, QL], f32r, tag=f"kh1T{e}", name=f"kh1T{e}")
                nc.vector.tensor_scalar_add(t, ps, bk1T[:, e:e + 1])
                kh1T.append(t)
            w1es.close()

            # CWT[j] = CT[j] * cqw[j]
            cwt = []
            for j in range(HT):
                tj = s1a.tile([P, CL], f32r, tag=f"cwt{j}", name=f"cwt{j}")
                nc.vector.tensor_scalar_mul(tj, ct[j].bitcast(f32),
                                            cqwT[:, j:j + 1])
                cwt.append(tj)

            # ---- s matrices ----
            s0_ps = smallp.tile([2, CL], f32, tag="smA", name="s0")
            for j in range(HT):
                nc.tensor.matmul(s0_ps, cwT[:, j, :], ct[j],
                                 start=(j == 0), stop=(j == HT - 1))
            s1_ps = smallp.tile([2, QL], f32, tag="smB", name="s1c")
            for j in range(HT):
                nc.tensor.matmul(s1_ps, qwT[:, j, :], qt[j],
                                 start=(j == 0), stop=(j == HT - 1))

            # augmented K=1 operands: sT += s1row x ones + ones x (s0+bias)
            s1row = s1a.tile([1, QL], f32r, tag="s1row", name="s1row")
            nc.vector.tensor_copy(s1row, s1_ps[0:1, :])
            ones64 = s1a.tile([1, QL], f32r, tag="ones64", name="ones64")
            nc.vector.memset(ones64.bitcast(f32), 1.0)
            s0brow = s1a.tile([1, CL], f32r, tag="s0brow", name="s0brow")
            nc.vector.tensor_scalar_add(s0brow, s0_ps[0:1, :],
                                        bias_sb[0:1, :])
            ones512 = s1a.tile([1, CL], f32r, tag="ones512", name="ones512")
            nc.vector.memset(ones512.bitcast(f32), 1.0)

            sT_ps = smallp.tile([QL, CL], f32, tag="smA", name="sT")
            for j in range(HT):
                nc.tensor.matmul(sT_ps, qt[j], cwt[j], start=(j == 0),
                                 stop=False)
            nc.tensor.matmul(sT_ps, s1row, ones512, start=False, stop=False)
            nc.tensor.matmul(sT_ps, ones64, s0brow, start=False, stop=True)
            s_qc = s1a.tile([QL, CL], f32, tag="s_qc", name="s_qc")
            nc.vector.tensor_copy(s_qc, sT_ps)

            # s2m in [q, c]
            s2m_qc = s1a.tile([QL, CL], f32r, tag="s2m_qc", name="s2m_qc")
            _msoftmax(nc, s1a, s_qc, s2m_qc, cm_b64, ncm_b64, QL, CL, "s2m")

            # s1m in [c, q]
            s1m_cq = []
            for i in range(CT_N):
                sc = s1a.tile([P, QL], f32, tag=f"s_cq{i}", name=f"s_cq{i}")
                nc.vector.tensor_copy(sc, pe_T(s_qc[:, i * P:(i + 1) * P]))
                sm = s1a.tile([P, QL], f32, tag=f"s1m_cq{i}",
                              name=f"s1m_cq{i}")
                _msoftmax(nc, s1a, sc, sm, qm_b, nqm_b, P, QL, f"s1m{i}")
                s1m_cq.append(sm)
            s1mT = s1b.tile([QL, CL], f32r, tag="s1mT", name="s1mT")
            for i in range(CT_N):
                nc.vector.tensor_copy(s1mT[:, i * P:(i + 1) * P],
                                      pe_T(s1m_cq[i]))

            # tT[d] [128d, 512c]
            tT_sb = []
            for d in range(CT_N):
                ps = smallp.tile([P, CL], f32, tag="smA", name="tT")
                nc.tensor.matmul(ps, s2m_qc[:, d * P:(d + 1) * P], s1mT,
                                 start=True, stop=True)
                t = s1b.tile([P, CL], f32r, tag=f"tT{d}", name=f"tT{d}")
                nc.vector.tensor_copy(t, ps)
                tT_sb.append(t)

            # ---- mha1 scores + scoat (accumulated unscaled: 4*scoat) ----
            def _sub(tiles, src_j, lo, width, tag):
                t = s1a.tile([64, width], f32r, tag=tag)
                nc.vector.tensor_copy(
                    t, tiles[src_j][lo:lo + 64, :].bitcast(f32))
                return t

            q_sub = {0: _sub(qh1T, 1, 0, CL, "qs0"),
                     1: _sub(qh1T, 1, 64, CL, "qs1"),
                     2: _sub(qh1T, 4, 0, CL, "qs2"),
                     3: _sub(qh1T, 4, 64, CL, "qs3")}
            k_sub = {0: _sub(kh1T, 1, 0, QL, "ks0"),
                     1: _sub(kh1T, 1, 64, QL, "ks1"),
                     2: _sub(kh1T, 4, 0, QL, "ks2"),
                     3: _sub(kh1T, 4, 64, QL, "ks3")}
            head_ops = {
                0: [(qh1T[0], kh1T[0]), (q_sub[0], k_sub[0])],
                1: [(q_sub[1], k_sub[1]), (qh1T[2], kh1T[2])],
                2: [(qh1T[3], kh1T[3]), (q_sub[2], k_sub[2])],
                3: [(q_sub[3], k_sub[3]), (qh1T[5], kh1T[5])],
            }

            scoat_cq = [s1a.tile([P, QL], f32, tag=f"scoat{i}",
                                 name=f"scoat{i}")
                        for i in range(CT_N)]
            for h in range(NHEAD1):
                for i in range(CT_N):
                    ps = smallp.tile([P, QL], f32, tag="smB", name="sc1")
                    ops = head_ops[h]
                    for ki, (ql, kr) in enumerate(ops):
                        nc.tensor.matmul(ps, ql[:, i * P:(i + 1) * P], kr,
                                         start=(ki == 0),
                                         stop=(ki == len(ops) - 1))
                    u = f"{h}_{i}"
                    e_sb = s1a.tile([P, QL], f32, tag=f"e1{u}", name=f"e1{u}")
                    ssum = s1a.tile([P, 1], f32, tag=f"ssum1{u}",
                                    name=f"ssum1{u}")
                    nc.scalar.activation(e_sb, ps, EXP, scale=ISQ,
                                         accum_out=ssum)
                    r = s1a.tile([P, 1], f32, tag=f"r1{u}", name=f"r1{u}")
                    nc.vector.reciprocal(r, ssum)
                    if h == 0:
                        nc.vector.tensor_scalar_mul(scoat_cq[i], e_sb, r)
                    else:
                        nc.vector.scalar_tensor_tensor(
                            scoat_cq[i], in0=e_sb, scalar=r,
                            in1=scoat_cq[i], op0=MULT, op1=ADD)

            # scoat1 -> scoat1T (f32r); mask folded with 1/4 scale
            scoat1T = s1b.tile([QL, CL], f32r, tag="scoat1T", name="scoat1T")
            for i in range(CT_N):
                sm = s1a.tile([P, QL], f32, tag=f"scoat1_{i}",
                              name=f"scoat1_{i}")
                _msoftmax(nc, s1a, scoat_cq[i], sm, qm_b4, nqm_b, P, QL,
                          f"sc1_{i}")
                nc.vector.tensor_copy(scoat1T[:, i * P:(i + 1) * P],
                                      pe_T(sm))

            # scoatT -> scoat2_qc -> scoat2_cq (f32r); 1/4 folded in mask
            scoatT = s1a.tile([QL, CL], f32, tag="scoatT", name="scoatT")
            for i in range(CT_N):
                nc.vector.tensor_copy(scoatT[:, i * P:(i + 1) * P],
                                      pe_T(scoat_cq[i]))
            scoat2_qc = s1a.tile([QL, CL], f32, tag="scoat2_qc",
                                 name="scoat2_qc")
            _msoftmax(nc, s1a, scoatT, scoat2_qc, cm_b64_4, ncm_b64,
                      QL, CL, "sc2")
            scoat2_cq = []
            for i in range(CT_N):
                t = s1b.tile([P, QL], f32r, tag=f"scoat2_cq{i}",
                             name=f"scoat2_cq{i}")
                nc.vector.tensor_copy(t,
                                      pe_T(scoat2_qc[:, i * P:(i + 1) * P]))
                scoat2_cq.append(t)
            s1es.close()  # free s1a pool, smallp (PSUM banks for bigp)
            bigp = s1bes.enter_context(
                tc.tile_pool(name="bigp", bufs=2, space="PSUM"))

            # bcoat [64q, 768h]
            bc_ps = bigp.tile([QL, H], f32, tag="big768", name="big768")
            for i in range(CT_N):
                nc.tensor.matmul(bc_ps[:, 0:512], scoat2_cq[i],
                                 crows[i][:, 0:512],
                                 start=(i == 0), stop=(i == CT_N - 1))
            for i in range(CT_N):
                nc.tensor.matmul(bc_ps[:, 512:H], scoat2_cq[i],
                                 crows[i][:, 512:H],
                                 start=(i == 0), stop=(i == CT_N - 1))
            bcoat = s1b.tile([QL, H], f32r, tag="bcoat", name="bcoat")
            nc.vector.tensor_copy(bcoat, bc_ps)

            # resident bf16 x / y and fp8 yT
            resp = es.enter_context(tc.tile_pool(name="resp", bufs=1,
                                                 side="right"))
            x_bf = [resp.tile([P, E2], bf16, tag=f"xbf{i}", name=f"xbf{i}")
                    for i in range(CT_N)]
            y_bf = [resp.tile([P, E2], bf16, tag=f"ybf{i}", name=f"ybf{i}")
                    for i in range(CT_N)]
            yT8 = resp.tile([P, ET, CL], f8, tag="yT8", name="yT8")

            trp2 = s1bes.enter_context(
                tc.tile_pool(name="trp2", bufs=2, space="PSUM"))
            gb_pool = s1bes.enter_context(tc.tile_pool(name="gb", bufs=1))
            scr_pool = s1bes.enter_context(tc.tile_pool(name="scr", bufs=2))
            gamma_b = gb_pool.tile([P, E2], bf16, tag="gamma_b",
                                   name="gamma_b")
            nc.sync.dma_start(out=gamma_b,
                              in_=d_gamma.ap().partition_broadcast(P))
            beta_b = gb_pool.tile([P, E2], bf16, tag="beta_b", name="beta_b")
            nc.sync.dma_start(out=beta_b,
                              in_=d_beta.ap().partition_broadcast(P))

            # ---- per-c-tile x assembly + LN (x, y stay resident bf16) ----
            pending_y = []
            for i in range(CT_N):
                x_i = x_bf[i]
                nc.vector.tensor_copy(x_i[:, XO_C:XO_C + H],
                                      crows[i].bitcast(f32))
                a_ps = bigp.tile([P, H], f32, tag="big768", name="big768")
                nc.tensor.matmul(a_ps[:, 0:512], s1mT[:, i * P:(i + 1) * P],
                                 qrows[:, 0:512], start=True, stop=True)
                nc.tensor.matmul(a_ps[:, 512:H], s1mT[:, i * P:(i + 1) * P],
                                 qrows[:, 512:H], start=True, stop=True)
                nc.scalar.copy(x_i[:, XO_A:XO_A + H], a_ps)
                nc.vector.tensor_mul(x_i[:, XO_CA:XO_CA + H],
                                     crows[i].bitcast(f32), a_ps)
                b_ps = bigp.tile([P, H], f32, tag="big768", name="big768")
                for d in range(CT_N):
                    nc.tensor.matmul(b_ps[:, 0:512],
                                     tT_sb[d][:, i * P:(i + 1) * P],
                                     crows[d][:, 0:512],
                                     start=(d == 0), stop=(d == CT_N - 1))
                for d in range(CT_N):
                    nc.tensor.matmul(b_ps[:, 512:H],
                                     tT_sb[d][:, i * P:(i + 1) * P],
                                     crows[d][:, 512:H],
                                     start=(d == 0), stop=(d == CT_N - 1))
                nc.vector.tensor_mul(x_i[:, XO_CB:XO_CB + H],
                                     crows[i].bitcast(f32), b_ps)
                s3_ps = bigp.tile([P, H], f32, tag="big768", name="big768")
                nc.tensor.matmul(s3_ps[:, 0:512],
                                 scoat1T[:, i * P:(i + 1) * P],
                                 bcoat[:, 0:512], start=True, stop=True)
                nc.tensor.matmul(s3_ps[:, 512:H],
                                 scoat1T[:, i * P:(i + 1) * P],
                                 bcoat[:, 512:H], start=True, stop=True)
                nc.scalar.copy(x_i[:, XO_S3:XO_S3 + H], s3_ps)
                ac_ps = bigp.tile([P, H], f32, tag="big768", name="big768")
                nc.tensor.matmul(ac_ps[:, 0:512],
                                 scoat1T[:, i * P:(i + 1) * P],
                                 qrows[:, 0:512], start=True, stop=True)
                nc.tensor.matmul(ac_ps[:, 512:H],
                                 scoat1T[:, i * P:(i + 1) * P],
                                 qrows[:, 512:H], start=True, stop=True)
                nc.scalar.copy(x_i[:, XO_AC:XO_AC + H], ac_ps)

                # layernorm
                stats = scr_pool.tile([P, 9, 6], f32, tag="stats",
                                      name="stats")
                xg = x_i.rearrange("p (g d) -> p g d", g=9)
                for g in range(9):
                    nc.vector.bn_stats(out=stats[:, g, :], in_=xg[:, g, :])
                mv = scr_pool.tile([P, 2], f32, tag="mv", name="mv")
                nc.vector.bn_aggr(out=mv, in_=stats)
                rsq = scr_pool.tile([P, 1], f32, tag="rsq", name="rsq")
                nc.scalar.activation(rsq, mv[:, 1:2], SQRT, bias=eps_sb,
                                     scale=1.0)
                rstd = scr_pool.tile([P, 1], f32, tag="rstd", name="rstd")
                nc.vector.reciprocal(rstd, rsq)
                negmr = scr_pool.tile([P, 1], f32, tag="negmr", name="negmr")
                nc.vector.tensor_scalar(negmr, mv[:, 0:1], rstd, -1.0,
                                        op0=MULT, op1=MULT)
                ytmp = scr_pool.tile([P, E2], bf16, tag="ytmp", name="ytmp")
                nc.scalar.activation(ytmp, x_i, IDENT, bias=negmr,
                                     scale=rstd)
                nc.vector.tensor_mul(ytmp, ytmp, gamma_b)
                nc.vector.tensor_add(y_bf[i], ytmp, beta_b)
                pending_y.append(i)
                if i > 0:
                    pi = pending_y.pop(0)
                    for j in range(ET):
                        nc.vector.tensor_copy(
                            yT8[:, j, pi * P:(pi + 1) * P],
                            pe_Tb(y_bf[pi][:, j * P:(j + 1) * P], trp2))
            for pi in pending_y:
                for j in range(ET):
                    nc.vector.tensor_copy(
                        yT8[:, j, pi * P:(pi + 1) * P],
                        pe_Tb(y_bf[pi][:, j * P:(j + 1) * P], trp2))
        # stage-1 pools freed (resp stays)

        # ================= phase 6: fp8 projections + scores + ss ========
        p56 = ExitStack()
        ssp = es.enter_context(tc.tile_pool(name="ssp", bufs=1))
        ss = [ssp.tile([P, CL], f32, tag=f"ss{i}", name=f"ss{i}")
              for i in range(CT_N)]
        with p56:
            prp = p56.enter_context(tc.tile_pool(name="prp", bufs=2))
            prps = p56.enter_context(
                tc.tile_pool(name="prps", bufs=3, space="PSUM"))
            scps = p56.enter_context(
                tc.tile_pool(name="scps", bufs=3, space="PSUM"))
            smp = p56.enter_context(tc.tile_pool(name="smp", bufs=4))

            bq2T = const.tile([P, ET], f32, tag="bq2T", name="bq2T")
            nc.sync.dma_start(out=bq2T,
                              in_=d_bq2.ap().rearrange("(t p) -> p t", p=P))
            bk2T = const.tile([P, ET], f32, tag="bk2T", name="bk2T")
            nc.sync.dma_start(out=bk2T,
                              in_=d_bk2.ap().rearrange("(t p) -> p t", p=P))

            for pair in range(NPAIR):
                load_pair_chunks(pair + 2)
                chunks_by_side = w_chunks.pop(pair)
                e0 = pair * 384
                projT = {}
                for side, bT in (("q", bq2T), ("k", bk2T)):
                    chunks = chunks_by_side[side]
                    pss = [prps.tile([P, CL], f32, tag=f"proj{e_}",
                                     name=f"proj{e_}", bufs=1)
                           for e_ in range(3)]
                    for u in range(ET // 2):
                        cki, t0 = divmod(2 * u, CH)
                        wt = chunks[cki]
                        for esub in range(3):
                            nc.tensor.matmul(
                                pss[esub],
                                wt[:, t0:t0 + 2, esub * P:(esub + 1) * P],
                                yT8[:, 2 * u:2 * u + 2, :],
                                start=(u == 0), stop=(u == ET // 2 - 1),
                                perf_mode=DR)
                    outs = []
                    for esub in range(3):
                        et_idx = (e0 // P) + esub
                        t = prp.tile([P, CL], bf16, tag=f"projT_{side}{esub}",
                                     name=f"projT_{side}{esub}", bufs=1)
                        nc.vector.tensor_scalar_add(
                            t, pss[esub], bT[:, et_idx:et_idx + 1])
                        outs.append(t)
                    lo = prp.tile([64, CL], bf16, tag=f"projlo{side}",
                                  name=f"projlo{side}", bufs=1)
                    nc.vector.tensor_copy(lo, outs[1][0:64, :])
                    hi = prp.tile([64, CL], bf16, tag=f"projhi{side}",
                                  name=f"projhi{side}", bufs=1)
                    nc.vector.tensor_copy(hi, outs[1][64:P, :])
                    projT[side] = (outs, lo, hi)

                qo, qlo, qhi = projT["q"]
                ko, klo, khi = projT["k"]
                for hh in range(2):
                    if hh == 0:
                        kops = [(qo[0], ko[0]), (qlo, klo)]
                    else:
                        kops = [(qhi, khi), (qo[2], ko[2])]
                    head_idx = pair * 2 + hh
                    for i in range(CT_N):
                        ps = scps.tile([P, CL], f32, tag="sc2", name="sc2")
                        for ki, (ql, kr) in enumerate(kops):
                            nc.tensor.matmul(ps, ql[:, i * P:(i + 1) * P],
                                             kr, start=(ki == 0),
                                             stop=(ki == 1))
                        e_sb = smp.tile([P, CL], f32, tag=f"e2_{i}",
                                        name=f"e2_{i}", bufs=2)
                        ssum = smp.tile([P, 1], f32, tag=f"ssum2_{i}",
                                        name=f"ssum2_{i}")
                        nc.scalar.activation(e_sb, ps, EXP, scale=ISQ,
                                             accum_out=ssum)
                        r = smp.tile([P, 1], f32, tag=f"r2_{i}",
                                     name=f"r2_{i}")
                        nc.vector.reciprocal(r, ssum)
                        if head_idx == 0:
                            nc.vector.tensor_scalar_mul(ss[i], e_sb, r)
                        else:
                            nc.vector.scalar_tensor_tensor(
                                ss[i], in0=e_sb, scalar=r,
                                in1=ss[i], op0=MULT, op1=ADD)
        # weight stream pool freed

        # ================= phase 7: ss1 + patt =================
        with ExitStack() as f7:
            fin = f7.enter_context(tc.tile_pool(name="fin", bufs=1))
            outp = f7.enter_context(tc.tile_pool(name="outp", bufs=3))
            pps = f7.enter_context(
                tc.tile_pool(name="pps", bufs=3, space="PSUM"))

            cm_b128 = const.tile([P, CL], f32, tag="cm_b128", name="cm_b128")
            nc.sync.dma_start(out=cm_b128,
                              in_=d_cm.ap().partition_broadcast(P))
            ncm_b128 = const.tile([P, CL], f32, tag="ncm_b128",
                                  name="ncm_b128")
            nc.sync.dma_start(out=ncm_b128,
                              in_=d_ncm.ap().partition_broadcast(P))

            ss1T = []
            for d in range(CT_N):
                sst = fin.tile([P, CL], f32, tag=f"ssT{d}", name=f"ssT{d}")
                for i in range(CT_N):
                    # 1/NHEAD2 scaling of ss folded into the transpose copy
                    nc.vector.tensor_scalar_mul(
                        sst[:, i * P:(i + 1) * P],
                        pe_T(ss[i][:, d * P:(d + 1) * P]), 1.0 / NHEAD2)
                t = fin.tile([P, CL], bf16, tag=f"ss1T{d}", name=f"ss1T{d}")
                _msoftmax(nc, fin, sst, t, cm_b128, ncm_b128, P, CL,
                          f"ss1_{d}")
                ss1T.append(t)

            for i in range(CT_N):
                for hs in range(E2 // 512):
                    ps = pps.tile([P, 512], f32, tag="patt", name="patt")
                    for d in range(CT_N):
                        nc.tensor.matmul(
                            ps, ss1T[d][:, i * P:(i + 1) * P],
                            y_bf[d][:, hs * 512:(hs + 1) * 512],
                            start=(d == 0), stop=(d == CT_N - 1))
                    o = outp.tile([P, 512], f32, tag="out", name="out")
                    nc.vector.tensor_add(o, ps,
                                         x_bf[i][:, hs * 512:(hs + 1) * 512])
                    nc.sync.dma_start(
                        out=d_out[i * P:(i + 1) * P,
                                  hs * 512:(hs + 1) * 512],
                        in_=o)

    nc.compile()
    return nc


# ================= host side =================

_CACHE = {}


def _pack_w8(w):
    """wq2/wk2 (E2_out, E2_in) f32 -> fp8 tiled (NPAIR*NCHUNK*P*CH, 384).

    Layout rows = [pair, cki, p, t] with contraction index
    k = (cki*CH + t)*P + p and output-feature column e in [0, 384) of
    block `pair`.
    """
    import ml_dtypes
    w8 = np.asarray(w, np.float32).astype(ml_dtypes.float8_e4m3)
    # w8[m, k]: m = pair*384 + e ; k = ((cki*6)+t)*128 + p
    w8 = w8.reshape(NPAIR, 384, NCHUNK, CH, P)     # [pair, e, cki, t, p]
    w8 = w8.transpose(0, 2, 4, 3, 1)               # [pair, cki, p, t, e]
    return np.ascontiguousarray(w8.reshape(NPAIR * NCHUNK * P * CH, 384))


def prep_shared(inputs):
    import ml_dtypes
    f = np.float32
    cw2 = np.zeros((768, 2), f)
    cw2[:, 0] = np.asarray(inputs["c_weight"], f).reshape(-1)
    qw2 = np.zeros((768, 2), f)
    qw2[:, 0] = np.asarray(inputs["q_weight"], f).reshape(-1)
    return {
        "cw2": cw2,
        "qw2": qw2,
        "cq_weight": np.ascontiguousarray(
            np.asarray(inputs["cq_weight"], f).reshape(-1)),
        "bias": np.ascontiguousarray(
            np.asarray(inputs["bias"], f).reshape(1, 1)),
        "wq1t": np.ascontiguousarray(np.asarray(inputs["wq1"], f).T),
        "wk1t": np.ascontiguousarray(np.asarray(inputs["wk1"], f).T),
        "bq1": np.ascontiguousarray(np.asarray(inputs["bq1"], f)),
        "bk1": np.ascontiguousarray(np.asarray(inputs["bk1"], f)),
        "gammab": np.asarray(inputs["gamma"], f).astype(ml_dtypes.bfloat16),
        "betab": np.asarray(inputs["beta"], f).astype(ml_dtypes.bfloat16),
        "wq2t8": _pack_w8(inputs["wq2"]),
        "wk2t8": _pack_w8(inputs["wk2"]),
        "bq2": np.ascontiguousarray(np.asarray(inputs["bq2"], f)),
        "bk2": np.ascontiguousarray(np.asarray(inputs["bk2"], f)),
    }


def make_in_maps(inputs, n_cores=8):
    f = np.float32
    shared = prep_shared(inputs)
    c = np.asarray(inputs["c"], f)
    q = np.asarray(inputs["q"], f)
    cm = np.asarray(inputs["c_mask"], f)
    qm = np.asarray(inputs["q_mask"], f)
    in_maps = []
    for b in range(n_cores):
        m = dict(shared)
        m["c"] = np.ascontiguousarray(c[b])
        m["q"] = np.ascontiguousarray(q[b])
        m["cm"] = np.ascontiguousarray(cm[b])
        m["ncm"] = np.ascontiguousarray((1.0 - cm[b]) * np.float32(NEG))
        m["cm4"] = np.ascontiguousarray(cm[b] * np.float32(0.25))
        m["qm"] = np.ascontiguousarray(qm[b])
        m["nqm"] = np.ascontiguousarray((1.0 - qm[b]) * np.float32(NEG))
        m["qm4"] = np.ascontiguousarray(qm[b] * np.float32(0.25))
        in_maps.append(m)
    return in_maps


def kernel(**inputs):
    from concourse.bass_utils import run_bass_kernel_spmd

    B = inputs["c"].shape[0]
    if "nc" not in _CACHE:
        _CACHE["nc"] = build(num_devices=B)
    nc = _CACHE["nc"]
    in_maps = make_in_maps(inputs, B)
    res = run_bass_kernel_spmd(nc, in_maps, core_ids=list(range(B)))
    out = np.stack([res.results[b]["out"] for b in range(B)])
    return out
